# revision 47
# baseline (speedup 1.0000x reference)
"""Trainium2 Bass kernel for nn_CCM: per-pixel complex 3x3 conv mask.

Math (per batch element b, sharded 1 batch element per NeuronCore):
  y[t,f] = sum_{c=0..26} m[c,t,f] * (w_{k(c)} * X)[t+i(c)-2, f+j(c)-1]
where c = 9*k + 3*i + j, w_k = v[0,k] + 1j*v[1,k] (cube roots of unity),
X = xr + 1j*xi, zero padded (causal in t: 2 top; symmetric in f: 1,1).

Design:
  - m streamed HBM->SBUF as fp16 via gpsimd SWDGE cast-DMA (free f32->f16
    convert inside the DMA engines, spreads across all 16 of them)
  - DVE computes only the 54 products per core in fp16 (2x_1P mode,
    ~1.23us per [125, 2056] op vs 2.3us for fp32)
  - TensorE accumulates products into PSUM via fp16 identity matmuls
    (start/stop accumulation groups), 4x512 cols per component
  - last 8 cols (tau=7, f>=249) accumulated on gpsimd in fp16 (PSUM is 16
    fp32/partition too small for 2x2056); gpsimd also paces the cast-DMAs
    one tap ahead of the tail reads
  - U planes (w_k * X) in fp16 with even/odd alignment copies so every
    tap slice is 4B-aligned (keeps the DVE 2x mode)
  - output transposed back on TensorE in fp16

Layout: t = 8*p + tau, partitions p in [0,125), (tau, f) in the free dim.
U planes are [125, 10 tau-slots, 260 f-cols] (slots tau=-2..7, f=-1..259).
"""

import sys
import numpy as np

sys.path.insert(0, "/opt/trn_rl_repo")

B = 8
C = 27
T = 1000
F = 257
TP = 125          # partitions
TAU = 8           # t = 8*p + tau
NS = 10           # tau slots in U planes: tau in [-2, 8)
FPX = 260         # padded f width (even, for fp16 alignment): f in [-1, 259)
NFREE = TAU * F   # 2056
MAIN = 2048       # PE-accumulated cols (4 psum banks x 512)
TAILW = NFREE - MAIN  # 8
TPAD = 1024       # padded t for the output staging
SQ3H = float(np.sqrt(3.0) / 2.0)
FCS = [(0, 128), (128, 128), (256, 1)]  # f chunks for transposes

_CACHE = {}


def _emit(ctx, tc, m_ap, x_ap, id32_ap, id16_ap, y_ap):
    import concourse.mybir as mybir

    nc = tc.nc
    f32 = mybir.dt.float32
    f16 = mybir.dt.float16

    const = ctx.enter_context(tc.tile_pool(name="const", bufs=1))
    planes = ctx.enter_context(tc.tile_pool(name="planes", bufs=1))
    mpool = ctx.enter_context(tc.tile_pool(name="mtiles", bufs=10))
    prpool = ctx.enter_context(tc.tile_pool(name="prod", bufs=4))

    ident = const.tile([128, 128], f32, tag="ident")
    nc.sync.dma_start(ident[:], id32_ap)
    ident16 = const.tile([128, 128], f16, tag="ident16")
    nc.sync.dma_start(ident16[:], id16_ap)

    # ---- U planes (fp16): xq_e = even-phase, xq_o = odd-phase (shift left 1)
    xq_e, xq_o = [], []
    for q in range(2):
        pe_ = planes.tile([TP, NS, FPX], f16, tag=f"xqe{q}", name="xqe")
        nc.vector.memset(pe_[:], 0.0)
        xq_e.append(pe_)
        po_ = planes.tile([TP, NS, FPX], f16, tag=f"xqo{q}", name="xqo")
        xq_o.append(po_)

    # ---- load x natural [f, (tt, comp)] (tt = t + 2), transpose into xq_e:
    # PE transpose (fp32) -> PSUM -> scalar copy (fp32->fp16) at fcol 1+f0
    with tc.tile_pool(name="psumx", bufs=3, space="PSUM") as psumx:
        xns = []
        for (f0, fw) in FCS:
            xn = const.tile([fw, (T + 2) * 2], f16, tag=f"xn{f0}", name="xn")
            nc.vector.memset(xn[:, 0:4], 0.0)
            nc.gpsimd.dma_start(
                xn[:, 4:], x_ap[f0:f0 + fw].rearrange("f t c -> f (t c)")
            )
            xns.append(xn)
        NSF = NS * FPX
        for q in range(2):
            for ts in range(NS):
                for ci, (f0, fw) in enumerate(FCS):
                    pt = psumx.tile([TP, 128], f16, tag="tp")
                    xn3 = xns[ci].rearrange("f (t c) -> f t c", c=2)
                    nc.tensor.transpose(
                        pt[0:TP, 0:fw],
                        xn3[0:fw, ts:ts + TAU * (TP - 1) + 1:TAU, q],
                        ident16[0:fw, 0:fw],
                    )
                    nc.scalar.copy(xq_e[q][:, ts, 1 + f0:1 + f0 + fw], pt[0:TP, 0:fw])
            # odd copy of this plane as soon as its slots are complete
            ef = xq_e[q].rearrange("p a b -> p (a b)")
            of = xq_o[q].rearrange("p a b -> p (a b)")
            nc.scalar.copy(of[:, 0:NSF - 1], ef[:, 1:NSF])

    # U_k = w_k * (xr + i xi), w_1/2 = -0.5 +- i*s : DVE fp16 (even phase).
    # tensor_scalar runs 4x and tensor_tensor 2x in fp16; scalar_tensor_tensor
    # has no 2x uop, so build from TS + TT.
    t1 = planes.tile([TP, NS, FPX], f16, tag="t1")
    t2 = planes.tile([TP, NS, FPX], f16, tag="t2")
    ta = planes.tile([TP, NS, FPX], f16, tag="ta")
    tb = planes.tile([TP, NS, FPX], f16, tag="tb")
    ur1e = planes.tile([TP, NS, FPX], f16, tag="ur1e")
    ui1e = planes.tile([TP, NS, FPX], f16, tag="ui1e")
    ur2e = planes.tile([TP, NS, FPX], f16, tag="ur2e")
    ui2e = planes.tile([TP, NS, FPX], f16, tag="ui2e")
    nc.vector.tensor_scalar_mul(t1[:], xq_e[1][:], SQ3H)   # s * xi
    nc.vector.tensor_scalar_mul(t2[:], xq_e[0][:], SQ3H)   # s * xr
    nc.vector.tensor_scalar_mul(ta[:], xq_e[0][:], -0.5)   # -xr/2
    nc.vector.tensor_scalar_mul(tb[:], xq_e[1][:], -0.5)   # -xi/2
    nc.vector.tensor_sub(ur1e[:], ta[:], t1[:])
    nc.vector.tensor_add(ui1e[:], tb[:], t2[:])
    nc.vector.tensor_add(ur2e[:], ta[:], t1[:])
    nc.vector.tensor_sub(ui2e[:], tb[:], t2[:])

    # odd copies of U1/U2 (scalar engine)
    ur1o = planes.tile([TP, NS, FPX], f16, tag="ur1o")
    ui1o = planes.tile([TP, NS, FPX], f16, tag="ui1o")
    ur2o = planes.tile([TP, NS, FPX], f16, tag="ur2o")
    ui2o = planes.tile([TP, NS, FPX], f16, tag="ui2o")
    for src, dst in ((ur1e, ur1o), (ui1e, ui1o), (ur2e, ur2o), (ui2e, ui2o)):
        sf = src.rearrange("p a b -> p (a b)")
        df_ = dst.rearrange("p a b -> p (a b)")
        nc.scalar.copy(df_[:, 0:NSF - 1], sf[:, 1:NSF])

    Ue = [(xq_e[0], xq_e[1]), (ur1e, ui1e), (ur2e, ui2e)]
    Uo = [(xq_o[0], xq_o[1]), (ur1o, ui1o), (ur2o, ui2o)]

    # fp16 accumulators for the 8-col tails (tau=7, f in [249,257))
    tail_r = planes.tile([TP, TAILW], f16, tag="tailr")
    tail_i = planes.tile([TP, TAILW], f16, tag="taili")
    # fp16 staging of the full accumulated result (for output transposes)
    acc16_r = planes.tile([TP, NFREE], f16, tag="acc16r")
    acc16_i = planes.tile([TP, NFREE], f16, tag="acc16i")

    # ---- tap loop: pr = m_c * U_sel (DVE fp16 2x); PE accumulates into PSUM
    # via identity matmuls. Tail cols accumulate on gpsimd, which also drives
    # the cast-DMAs: pool order is gen(0..2), gen(3), tails(0), gen(4),
    # tails(1), ... so the DMA stream stays ~3 taps ahead.
    with tc.tile_pool(name="psacc", bufs=1, space="PSUM") as psacc:
        accR = [psacc.tile([TP, 512], f32, tag=f"aR{j}", name=f"aR{j}")
                for j in range(4)]
        accI = [psacc.tile([TP, 512], f32, tag=f"aI{j}", name=f"aI{j}")
                for j in range(4)]

        def issue_m(c):
            mt = mpool.tile([TP, NFREE], f16, tag="mt", name="mt")
            nc.gpsimd.dma_start(mt[:], m_ap[c].rearrange("(p t) f -> p (t f)", p=TP))
            return mt

        def accum(psum_chunks, pr, start, stop):
            for jc in range(4):
                nc.tensor.matmul(
                    psum_chunks[jc][:, :], ident16[0:TP, 0:TP],
                    pr[:, 512 * jc:512 * (jc + 1)],
                    start=start, stop=stop,
                )

        AHEAD = 10
        mts = [issue_m(c) for c in range(AHEAD)]
        for c in range(C):
            kk, n = divmod(c, 9)
            i, j = divmod(n, 3)
            dt, df = i - 2, j - 1
            if df == 0:
                ur, ui = Uo[kk]
                fc = 0
            else:
                ur, ui = Ue[kk]
                fc = df + 1
            urs = ur[:, dt + 2:dt + 2 + TAU, fc:fc + F]
            uis = ui[:, dt + 2:dt + 2 + TAU, fc:fc + F]
            m3 = mts[c].rearrange("p (t f) -> p t f", f=F)
            start, stop = (c == 0), (c == C - 1)

            pr = prpool.tile([TP, NFREE], f16, tag="pr")
            pr3 = pr.rearrange("p (t f) -> p t f", f=F)
            nc.vector.tensor_mul(pr3[:], m3[:], urs)
            accum(accR, pr, start, stop)

            pi = prpool.tile([TP, NFREE], f16, tag="pr")
            pi3 = pi.rearrange("p (t f) -> p t f", f=F)
            nc.vector.tensor_mul(pi3[:], m3[:], uis)
            accum(accI, pi, start, stop)

            # pool engine: prefetch m tile c+AHEAD, then tail-adds for tap c
            if c + AHEAD < C:
                mts.append(issue_m(c + AHEAD))
            if c == 0:
                nc.gpsimd.tensor_copy(tail_r[:], pr[:, MAIN:NFREE])
                nc.gpsimd.tensor_copy(tail_i[:], pi[:, MAIN:NFREE])
            else:
                nc.gpsimd.tensor_add(tail_r[:], tail_r[:], pr[:, MAIN:NFREE])
                nc.gpsimd.tensor_add(tail_i[:], tail_i[:], pi[:, MAIN:NFREE])

        # evacuate PSUM (fp32) -> SBUF fp16 staging (scalar + DVE in parallel)
        for jc in range(4):
            nc.scalar.copy(acc16_r[:, 512 * jc:512 * (jc + 1)], accR[jc][:, :])
            nc.vector.tensor_copy(acc16_i[:, 512 * jc:512 * (jc + 1)],
                                  accI[jc][:, :])
        nc.scalar.copy(acc16_r[:, MAIN:NFREE], tail_r[:])
        nc.vector.tensor_copy(acc16_i[:, MAIN:NFREE], tail_i[:])

    # ---- transpose back to [f, (t, comp)] (fp16 PE transposes) and store
    acc3_r = acc16_r.rearrange("p (t f) -> p t f", f=F)
    acc3_i = acc16_i.rearrange("p (t f) -> p t f", f=F)
    with tc.tile_pool(name="psumo", bufs=4, space="PSUM") as psumo:
        ncp = 0
        for ci, (f0, fw) in enumerate(FCS):
            yo = const.tile([fw, T * 2], f32, tag=f"yo{f0}", name="yo")
            yv = yo.rearrange("f (t c) -> f t c", c=2)
            for comp, acc in ((0, acc3_r), (1, acc3_i)):
                for ts in range(TAU):
                    pt = psumo.tile([128, TP], f16, tag="tp2")
                    nc.tensor.transpose(
                        pt[0:fw, 0:TP], acc[:, ts, f0:f0 + fw], ident16[0:TP, 0:TP]
                    )
                    dst = yv[0:fw, ts:ts + TAU * (TP - 1) + 1:TAU, comp]
                    if ncp % 2 == 0:
                        nc.scalar.copy(dst, pt[0:fw, 0:TP])
                    else:
                        nc.vector.tensor_copy(dst, pt[0:fw, 0:TP])
                    ncp += 1
            # spread the writeback across the three DMA-issuing engines
            ydst = y_ap[f0:f0 + fw].rearrange("f t c -> f (t c)")
            if ci == 0:
                nc.sync.dma_start(ydst, yo[:])
            elif ci == 1:
                nc.gpsimd.dma_start(ydst, yo[:])
            else:
                nc.scalar.dma_start(ydst, yo[:])


def _build():
    if "nc" in _CACHE:
        return _CACHE["nc"]
    from contextlib import ExitStack
    from concourse import bacc, mybir
    import concourse.tile as tile

    f32 = mybir.dt.float32
    f16 = mybir.dt.float16
    nc = bacc.Bacc("TRN2", target_bir_lowering=False, debug=False, num_devices=B)
    m_d = nc.dram_tensor("m", (C, T, F), f32, kind="ExternalInput")
    x_d = nc.dram_tensor("x", (F, T, 2), f32, kind="ExternalInput")
    id32_d = nc.dram_tensor("ident", (128, 128), f32, kind="ExternalInput")
    id16_d = nc.dram_tensor("ident16", (128, 128), f16, kind="ExternalInput")
    y_d = nc.dram_tensor("y", (F, T, 2), f32, kind="ExternalOutput")

    with tile.TileContext(nc) as tc:
        with ExitStack() as ctx:
            _emit(ctx, tc, m_d.ap(), x_d.ap(), id32_d.ap(), id16_d.ap(), y_d.ap())
    nc.compile()
    _CACHE["nc"] = nc
    return nc


def _in_maps(m, x):
    ident = np.eye(128, dtype=np.float32)
    ident16 = np.eye(128, dtype=np.float16)
    return [
        {"m": np.ascontiguousarray(m[b]), "x": np.ascontiguousarray(x[b]),
         "ident": ident, "ident16": ident16}
        for b in range(B)
    ]


def kernel(m, x, v, _trace=False):
    from concourse import bass_utils

    m = np.asarray(m, dtype=np.float32)
    x = np.asarray(x, dtype=np.float32)
    nc = _build()
    res = bass_utils.run_bass_kernel_spmd(
        nc, _in_maps(m, x), core_ids=list(range(B)), trace=_trace
    )
    kernel.last_results = res
    y = np.stack([res.results[b]["y"] for b in range(B)], axis=0)
    return y
